# revision 24
# baseline (speedup 1.0000x reference)
"""Trainium2 kernel: per-pixel channel-mixing attention via temperature
interpolation (sigma-interp), v3.

Math per pixel: out_i = sum_j sigma_i(k_j) q_j where sigma(t) = softmax(t*v)
over channels. sigma(t*v) is interpolated in the temperature t at M=11
per-pixel-scaled Chebyshev nodes t_m = Tk*u_m (Tk = max|k| per pixel):

    out_i = sum_m exp(u_m * Tk*v_i) * S_m,   S_m = R_m / G_m
    G_m   = sum_i exp(u_m * Tk*v_i)
    R_m   = sum_r Lc[r,m] * That_r,  That_r = sum_j T_r(k_j/Tk) q_j

M=11 includes the center node u=0 whose grid is identically 1: no exp pass,
no eval multiply (G via a ones-tile reduce; eval contribution is a single
broadcast-accumulate matmul pair into the output).

Host sorts pixels by A = max|v|*max|k|; the hardest 128 per core go through
an exact pair-grid tile.

v3 perf structure:
  - analysis reductions 4-way concurrent on the PE (h0 G/That at PSUM rows
    0/32, h1 at 64/96 -> four col_grp quadrants, one stationary pattern).
  - eval broadcast/accumulate matmuls split 64/64 by channel group for
    2-way col_grp concurrency.
  - chebyshev chain parity-split over z = 2*khat^2-1: even/odd subchains
    halve the serial dependency depth.
  - exact-tile row/col sums on the PE via stride-0 output access patterns.
  - PE warmup + junk filler matmuls keep the HAM clock gate at 2.4 GHz.
  - no Ln activations (reciprocal_approx_fast on DVE): exp table loads once.
  - eval S tiles: PE bcast -> fp16 SBUF copy (ACT) -> 2x DVE multiply for
    most nodes; a few nodes read PSUM directly on DVE to balance engines.
"""

import sys

sys.path.insert(0, "/opt/trn_rl_repo")

from contextlib import ExitStack

import ml_dtypes
import numpy as np

import concourse.bacc as bacc
import concourse.bass as bass
import concourse.tile as tile
from concourse import mybir
from concourse.bass_utils import run_bass_kernel_spmd

B, C, H, W = 2, 64, 128, 128
N_CORES = 8
NPIX = B * H * W            # 32768
M = 11                      # interp nodes (odd: center node u=0 is free)
MC = M // 2
NEX_CORE = 128              # exact pixels per core
NEZ_CORE = NPIX // N_CORES - NEX_CORE   # 3968 interp pixels per core
FD = NEZ_CORE // 2          # 1984 pixels per g-half
HFD = FD // 2               # 992 pixels per column-half
R2 = 2 * M
N_WARM = 20                 # PE warmup matmuls
N_JUNK_ANA = 4              # filler matmuls per analysis m-step
# (half, m) slots whose S stays in PSUM (DVE 1x mul, no ACT copy)
PSUM_MUL_SLOTS = {(0, 2), (1, 2), (0, 8), (1, 8)}

FP32 = mybir.dt.float32
FP16 = mybir.dt.float16
BF16 = mybir.dt.bfloat16
EXP = mybir.ActivationFunctionType.Exp

U_NODES = np.cos(np.pi * np.arange(M) / (M - 1))


def _lc_matrix():
    u = U_NODES
    Tn = np.cos(np.arange(M)[:, None] * np.arccos(np.clip(u, -1, 1))[None, :])
    return np.linalg.inv(Tn.T)


def build_kernel():
    nc = bacc.Bacc(
        "TRN2",
        target_bir_lowering=False,
        debug=False,
        enable_asserts=False,
        num_devices=N_CORES,
    )
    vp = nc.dram_tensor("vp", [128, FD], FP16, kind="ExternalInput").ap()
    k2t = nc.dram_tensor("k2t", [128, FD], FP16, kind="ExternalInput").ap()
    qt = nc.dram_tensor("qt", [128, FD], FP16, kind="ExternalInput").ap()
    u1t = nc.dram_tensor("u1t", [128, FD], FP16, kind="ExternalInput").ap()
    statR = nc.dram_tensor("statR", [128, M, R2], FP16, kind="ExternalInput").ap()
    statL = nc.dram_tensor("statL", [128, M, R2], FP16, kind="ExternalInput").ap()
    statB = nc.dram_tensor("statB", [R2, M, 128], FP16, kind="ExternalInput").ap()
    identb = nc.dram_tensor("identb", [128, 128], BF16, kind="ExternalInput").ap()
    ident2 = nc.dram_tensor("ident2", [128, 64], FP16, kind="ExternalInput").ap()
    vE = nc.dram_tensor("vE", [128, C], FP32, kind="ExternalInput").ap()
    kE = nc.dram_tensor("kE", [128, C], FP32, kind="ExternalInput").ap()
    qE = nc.dram_tensor("qE", [128, C], FP32, kind="ExternalInput").ap()

    outm = nc.dram_tensor("outm", [128, FD], FP16, kind="ExternalOutput").ap()
    oute = nc.dram_tensor("oute", [128, C], FP32, kind="ExternalOutput").ap()

    CH = [0, 512, HFD]

    with tile.TileContext(nc) as tc, ExitStack() as ctx:
        sb = ctx.enter_context(tc.tile_pool(name="sb", bufs=1))
        sbw = ctx.enter_context(tc.tile_pool(name="sbw", bufs=1))
        sbp = ctx.enter_context(tc.tile_pool(name="sbp", bufs=2))
        acp = ctx.enter_context(tc.tile_pool(name="acp", bufs=2, space="PSUM"))

        # ---- PE warmup: junk matmuls from t~0 flip HAM to 2.4 GHz ----
        wu = sbw.tile([128, 512], FP16, tag="wu")
        nc.vector.memset(wu, 1.0)
        wu_ps = acp.tile([128, HFD], FP32, tag="acc", name="wups")
        for i in range(N_WARM):
            nc.tensor.matmul(wu_ps[:, 0:496], wu[:, 0:128], wu[:, 0:496],
                             start=True, stop=True)
        accs = [acp.tile([128, HFD], FP32, tag="acc", name=f"acc{h}")
                for h in range(2)]
        junk_tgts = []
        _junk_i = [0]

        def junk_mm(n):
            # filler matmuls into unused partition rows of live psum tiles
            # (same bank, disjoint partitions -> safe): keeps the PE activity
            # monitor busy so the clock gate stays at 2.4 GHz
            for _ in range(n):
                t = junk_tgts[_junk_i[0] % len(junk_tgts)]
                _junk_i[0] += 1
                nc.tensor.matmul(t[64:128, 0:496], wu[:, 0:64], wu[:, 0:496],
                                 start=True, stop=True, skip_group_check=True)

        # dummy activation: pull the exp table load off the critical path
        dum = sbw.tile([1, 16], FP32, tag="dum")
        nc.scalar.activation(out=dum, in_=wu[0:1, 0:16], func=EXP)

        # ---- input DMAs ----
        v_t = sb.tile([128, FD], FP16)
        k2_t = sb.tile([128, FD], FP16)
        q_t = sb.tile([128, FD], FP16)
        u1_t = sb.tile([128, FD], FP16)
        sR = sb.tile([128, M, R2], FP16)
        sL = sb.tile([128, M, R2], FP16)
        sB_t = sb.tile([R2, M, 128], FP16)
        id_t = sb.tile([128, 128], BF16)
        id2_t = sb.tile([128, 64], FP16)
        nc.sync.dma_start(out=v_t[:, :FD // 2], in_=vp[:, :FD // 2])
        nc.sync.dma_start(out=k2_t, in_=k2t)
        nc.sync.dma_start(out=u1_t, in_=u1t)
        nc.sync.dma_start(out=q_t, in_=qt)
        nc.sync.dma_start(out=v_t[:, FD // 2:], in_=vp[:, FD // 2:])
        nc.gpsimd.dma_start(out=sR, in_=statR)
        nc.gpsimd.dma_start(out=sL, in_=statL)
        nc.gpsimd.dma_start(out=sB_t, in_=statB)
        nc.gpsimd.dma_start(out=id_t, in_=identb)
        nc.gpsimd.dma_start(out=id2_t, in_=ident2)

        ones_t = sbw.tile([128, FD], FP16, tag="ones")
        nc.vector.memset(ones_t, 1.0)

        # ---- X grids (ACT, fp16) ----
        Xs = {}
        for m in range(M):
            if m == MC:
                Xs[m] = ones_t
                continue
            xm = sbw.tile([128, FD], FP16, tag=f"x{m}")
            nc.scalar.activation(out=xm[:, :FD // 2], in_=v_t[:, :FD // 2],
                                 func=EXP, scale=float(U_NODES[m]))
            nc.scalar.activation(out=xm[:, FD // 2:], in_=v_t[:, FD // 2:],
                                 func=EXP, scale=float(U_NODES[m]))
            Xs[m] = xm

        # ---- chebyshev chain, parity split over zz = 2z = k2^2 - 2 ----
        # U_r = T_r(khat) * q.  evens: E_s = T_s(z) q = U_{2s};
        # odds: O_s = cos((2s+1)arccos(khat)) q = U_{2s+1};
        # both satisfy  next = zz * cur - prev.
        zz = sbw.tile([128, FD], FP16, tag="zz")
        nc.vector.tensor_mul(zz, k2_t, k2_t)
        nc.vector.tensor_scalar_add(zz, zz, -2.0)
        Us = {0: q_t, 1: u1_t}
        e1 = sbw.tile([128, FD], FP16, tag="e1")
        nc.vector.scalar_tensor_tensor(
            out=e1, in0=zz, scalar=0.5, in1=q_t,
            op0=mybir.AluOpType.mult, op1=mybir.AluOpType.mult)
        Us[2] = e1
        o1 = sbw.tile([128, FD], FP16, tag="o1")
        nc.vector.scalar_tensor_tensor(
            out=o1, in0=zz, scalar=-1.0, in1=u1_t,
            op0=mybir.AluOpType.add, op1=mybir.AluOpType.mult)
        Us[3] = o1
        for par in (0, 1):
            prev, cur = Us[0 + par], Us[2 + par]
            for r in range(4 + par, M, 2):
                tmp = sbp.tile([128, FD], FP16, tag=f"tmp{par}")
                nc.vector.tensor_mul(tmp, zz, cur)
                nxt = sbw.tile([128, FD], FP16, tag=f"u{r}")
                nc.vector.tensor_sub(nxt, tmp, prev)
                Us[r] = nxt
                prev, cur = cur, nxt

        # ---- analysis reductions, 4-way col_grp concurrent ----
        # one psum tile: h0 G rows 0:22, h0 That rows 32:54,
        #                h1 G rows 64:86, h1 That rows 96:118
        with tc.tile_pool(name="red", bufs=2, space="PSUM") as redp:
            red_ts = [redp.tile([128, HFD], FP32, tag="red", name=f"red{h}")
                      for h in range(2)]
            junk_tgts.extend(red_ts)
            for m in range(M):
                for h in range(2):
                    hs = slice(h * HFD, (h + 1) * HFD)
                    r_ps = red_ts[h][0:R2, :]
                    g_ps = red_ts[h][32:32 + R2, :]
                    for a, b in zip(CH[:-1], CH[1:]):
                        nc.tensor.matmul(r_ps[:, a:b], sL[:, m, :],
                                         Us[m][:, hs][:, a:b],
                                         start=(m == 0), stop=(m == M - 1))
                        nc.tensor.matmul(g_ps[:, a:b], sR[:, m, :],
                                         Xs[m][:, hs][:, a:b],
                                         start=(m == 0), stop=(m == M - 1))
                junk_mm(N_JUNK_ANA)

            # ---- S = R / G per half (R = Lc-weighted That, direct) ----
            s_halves = []
            for h in range(2):
                r_ps = red_ts[h][0:R2, :]
                g_ps = red_ts[h][32:32 + R2, :]
                gsb = sbw.tile([R2, HFD], FP32, tag=f"gsb{h}")
                nc.scalar.copy(gsb, g_ps)
                junk_mm(7)
                ginv = sbw.tile([R2, HFD], FP32, tag=f"ginv{h}")
                nc.vector.reciprocal_approx_fast(out=ginv, in_=gsb)
                s_th = sbw.tile([R2, HFD], FP16, tag=f"s{h}")
                nc.vector.tensor_mul(s_th, ginv, r_ps)
                s_halves.append(s_th)

            # ---- exact tile (pixel-major pair-grid; sums on the PE) ----
            v2 = sb.tile([128, C, 2], FP16)
            nc.scalar.copy(v2, vE_bcast(nc, sb, vE))
            kE16 = sb.tile([128, C], FP16)
            qE_t = sb.tile([128, C], FP32)
            nc.sync.dma_start(out=qE_t, in_=qE)
            nc.scalar.copy(kE16, kE_load(nc, sb, kE))
            P_t = sb.tile([128, C, C], FP16)
            k_op = bass.AP(
                tensor=kE16.tensor, offset=kE16.offset,
                ap=[kE16.ap[0], [0, C], [2, C // 2], [1, 2]],
            )
            v_op = bass.AP(
                tensor=v2.tensor, offset=v2.offset,
                ap=[v2.ap[0], [2, C], [0, C // 2], [1, 2]],
            )
            nc.vector.tensor_mul(
                P_t.rearrange("p i (jh jp) -> p i jh jp", jp=2), k_op, v_op)
            E_t = sb.tile([128, C, C], BF16)
            nc.scalar.activation(out=E_t, in_=P_t, func=EXP)
            G1 = sb.tile([128, C // 4, C], BF16)
            G2 = sb.tile([128, C // 4, C], BF16)
            nc.vector.tensor_add(G1, E_t[:, : C // 4, :], E_t[:, C // 4: C // 2, :])
            nc.vector.tensor_add(G2, E_t[:, C // 2: 3 * C // 4, :], E_t[:, 3 * C // 4:, :])
            nc.gpsimd.dma_start(out=G1, in_=G2, accum_op=mybir.AluOpType.add)
            nc.vector.tensor_add(G1[:, : C // 8, :], G1[:, : C // 8, :],
                                 G1[:, C // 8: C // 4, :])
            nc.vector.tensor_add(G1[:, : C // 16, :], G1[:, : C // 16, :],
                                 G1[:, C // 16: C // 8, :])
            d_t = sb.tile([128, C], FP32)
            nc.vector.tensor_reduce(
                out=d_t, in_=G1[:, : C // 16, :].transpose([0, 2, 1]),
                axis=mybir.AxisListType.X, op=mybir.AluOpType.add,
            )
            r_t = sb.tile([128, C], FP32)
            nc.vector.reciprocal_approx_fast(out=r_t, in_=d_t)
            w16 = sb.tile([128, C], BF16)
            nc.vector.tensor_mul(w16, qE_t, r_t)
            Q4 = C // 4
            F1 = sb.tile([128, C, Q4], BF16)
            F2 = sb.tile([128, C, Q4], BF16)
            F3 = sb.tile([128, C, Q4], BF16)
            F4 = sb.tile([128, C, Q4], BF16)
            for fi, Fq in enumerate((F1, F2, F3, F4)):
                nc.vector.tensor_mul(
                    Fq, E_t[:, :, fi * Q4: (fi + 1) * Q4],
                    w16[:, None, fi * Q4: (fi + 1) * Q4].broadcast_to([128, C, Q4]),
                )
            nc.gpsimd.dma_start(out=F1, in_=F2, accum_op=mybir.AluOpType.add)
            nc.gpsimd.dma_start(out=F3, in_=F4, accum_op=mybir.AluOpType.add)
            nc.vector.tensor_add(F1, F1, F3)
            nc.vector.tensor_add(F1[:, :, : Q4 // 2], F1[:, :, : Q4 // 2],
                                 F1[:, :, Q4 // 2:])
            nc.vector.tensor_add(F1[:, :, : Q4 // 4], F1[:, :, : Q4 // 4],
                                 F1[:, :, Q4 // 4: Q4 // 2])
            oE = sb.tile([128, C], FP32)
            nc.vector.tensor_reduce(
                out=oE, in_=F1[:, :, : Q4 // 4],
                axis=mybir.AxisListType.X, op=mybir.AluOpType.add,
            )
            nc.sync.dma_start(out=oute, in_=oE)

        # ---- eval: out = sum_m X_m * bcast(S_m) (red pool released) ----
        # 4-deep chunk-level pipeline: s_b [128,496] x4 psum bufs; broadcast
        # and accumulate matmuls split by channel group for array pairing.
        with tc.tile_pool(name="evp", bufs=3, space="PSUM") as evp, \
             tc.tile_pool(name="jnk", bufs=1, space="PSUM") as jnkp:
            jnk_t = jnkp.tile([128, 496], FP32, tag="jnk")
            junk_tgts.clear()

            def junk_ev(n):
                for _ in range(n):
                    nc.tensor.matmul(jnk_t[:, 0:496], wu[:, 0:128],
                                     wu[:, 0:496], start=True, stop=True,
                                     skip_group_check=True)
            for m in range(M):
                for half in range(2):
                    acc = accs[half]
                    s_h = s_halves[half]
                    if m == MC:
                        for a, b in zip(CH[:-1], CH[1:]):
                            nc.tensor.matmul(acc[0:64, a:b], sB_t[:, m, 0:64],
                                             s_h[:, a:b], start=False, stop=False,
                                             skip_group_check=True)
                            nc.tensor.matmul(acc[64:128, a:b], sB_t[:, m, 64:128],
                                             s_h[:, a:b], start=False, stop=False,
                                             skip_group_check=True)
                        continue
                    st = (m == 0)
                    sp = (m == M - 1)
                    for ci, (a, b) in enumerate(zip(CH[:-1], CH[1:])):
                        w = b - a
                        s_b = evp.tile([128, 512], FP32, tag="sbps",
                                       name=f"sb{half}_{m}_{ci}")
                        nc.tensor.matmul(s_b[0:64, 0:w], sB_t[:, m, 0:64],
                                         s_h[:, a:b], start=True, stop=True)
                        nc.tensor.matmul(s_b[64:128, 0:w], sB_t[:, m, 64:128],
                                         s_h[:, a:b], start=True, stop=True)
                        prod = sbp.tile([128, 512], FP16, tag="prod", bufs=6)
                        xs = Xs[m][:, half * HFD + a: half * HFD + b]
                        if (half, m) in PSUM_MUL_SLOTS:
                            nc.vector.tensor_mul(prod[:, 0:w], xs, s_b[:, 0:w])
                        else:
                            s_bs = sbp.tile([128, 512], FP16, tag="sbs", bufs=6)
                            nc.scalar.copy(s_bs[:, 0:w], s_b[:, 0:w])
                            nc.vector.tensor_mul(prod[:, 0:w], xs, s_bs[:, 0:w])
                        nc.tensor.matmul(acc[0:64, a:b], id2_t[0:64, :],
                                         prod[0:64, 0:w], start=st, stop=sp,
                                         skip_group_check=True)
                        nc.tensor.matmul(acc[64:128, a:b], id2_t[64:128, :],
                                         prod[64:128, 0:w], start=st, stop=sp,
                                         skip_group_check=True,
                                         tile_position=(64, 64))
                        junk_ev(1)
            for half in range(2):
                o_sb = sbp.tile([128, HFD], FP16, tag="osb")
                nc.scalar.copy(o_sb, accs[half][:, 0:HFD])
                nc.sync.dma_start(
                    out=outm[:, half * HFD:(half + 1) * HFD], in_=o_sb)

    nc.compile()
    return nc


def vE_bcast(nc, sb, vE):
    vE_t = sb.tile([128, C], FP32)
    nc.sync.dma_start(out=vE_t, in_=vE)
    return vE_t[:, :, None].broadcast_to([128, C, 2])


def kE_load(nc, sb, kE):
    kE_t = sb.tile([128, C], FP32)
    nc.sync.dma_start(out=kE_t, in_=kE)
    return kE_t


_NC_CACHE = None


def _get_nc():
    global _NC_CACHE
    if _NC_CACHE is None:
        _NC_CACHE = build_kernel()
    return _NC_CACHE


def _prep(x, y, z):
    """Host prep: sort by difficulty, shard, scale. Returns in_maps + meta."""
    q = np.ascontiguousarray(np.transpose(np.asarray(x), (0, 2, 3, 1))).reshape(-1, C)
    k = np.ascontiguousarray(np.transpose(np.asarray(y), (0, 2, 3, 1))).reshape(-1, C)
    v = np.ascontiguousarray(np.transpose(np.asarray(z), (0, 2, 3, 1))).reshape(-1, C)
    Tk = np.abs(k).max(axis=1)
    A = Tk * np.abs(v).max(axis=1)
    order = np.argsort(A, kind="stable")
    easy = order[: NEZ_CORE * N_CORES]
    hard = order[NEZ_CORE * N_CORES:]

    Lc = _lc_matrix()
    statR = np.zeros((128, M, R2), np.float32)
    for m in range(M):
        for g in range(2):
            statR[g * 64:(g + 1) * 64, m, 2 * m + g] = 1
    statL = np.zeros((128, M, R2), np.float32)
    for r in range(M):
        for m in range(M):
            for g in range(2):
                statL[g * 64:(g + 1) * 64, r, 2 * m + g] = Lc[r, m]
    statB = np.zeros((R2, M, 128), np.float32)
    for m in range(M):
        for g in range(2):
            statB[2 * m + g, m, g * 64:(g + 1) * 64] = 1
    identb = np.eye(128, dtype=np.float32)
    ident2 = np.zeros((128, 64), np.float32)
    ident2[0:64] = np.eye(64)
    ident2[64:128] = np.eye(64)

    in_maps = []
    meta = []
    for c in range(N_CORES):
        ez = easy[c::N_CORES]
        hd = hard[c::N_CORES]
        kh = k[ez] / Tk[ez, None]
        vp_c = (Tk[ez, None] * v[ez]).astype(np.float16)
        k2_c = (2.0 * kh).astype(np.float16)
        q_c = q[ez].astype(np.float16)
        u1_c = (kh * q[ez]).astype(np.float16)

        def cmaj(a2d, dt):
            h0 = a2d[:FD].T
            h1 = a2d[FD:].T
            return np.ascontiguousarray(np.concatenate([h0, h1], axis=0)).astype(dt)

        in_maps.append({
            "vp": cmaj(vp_c, np.float16),
            "k2t": cmaj(k2_c, np.float16),
            "qt": cmaj(q_c, np.float16),
            "u1t": cmaj(u1_c, np.float16),
            "statR": statR.astype(np.float16),
            "statL": statL.astype(np.float16),
            "statB": statB.astype(np.float16),
            "identb": identb.astype(ml_dtypes.bfloat16),
            "ident2": ident2.astype(np.float16),
            "vE": v[hd].astype(np.float32),
            "kE": k[hd].astype(np.float32),
            "qE": q[hd].astype(np.float32),
        })
        meta.append((ez, hd))
    return in_maps, meta


def kernel(x, y, z):
    nc = _get_nc()
    in_maps, meta = _prep(x, y, z)
    res = run_bass_kernel_spmd(nc, in_maps, core_ids=list(range(N_CORES)))
    out = np.empty((NPIX, C), np.float32)
    for c in range(N_CORES):
        ez, hd = meta[c]
        om = res.results[c]["outm"].astype(np.float32)
        out[ez[:FD]] = om[:64].T
        out[ez[FD:]] = om[64:].T
        out[hd] = res.results[c]["oute"]
    return np.ascontiguousarray(
        np.transpose(out.reshape(B, H, W, C), (0, 3, 1, 2))
    ).astype(np.float32)


# revision 26
# speedup vs baseline: 1.3063x; 1.3063x over previous
"""Trainium2 kernel: per-pixel channel-mixing attention via temperature
interpolation (sigma-interp), v7.

Math per pixel: out_i = sum_j sigma_i(k_j) q_j where sigma(t) = softmax(t*v)
over channels. sigma(t*v) is interpolated in the temperature t at M=11
per-pixel-scaled Chebyshev nodes t_m = Tk*u_m (Tk = max|k| per pixel):

    out_i = sum_m exp(u_m * Tk*v_i) * S_m,   S_m = R_m / G_m
    G_m   = sum_i exp(u_m * Tk*v_i)
    R_m   = sum_r Lc[r,m] * That_r,  That_r = sum_j T_r(k_j/Tk) q_j

M=11 includes the center node u=0 whose grid is identically 1: no exp pass,
no eval multiply (G via a ones-tile reduce; its eval contribution is a
broadcast-accumulate matmul pair directly into the output).

Host sorts pixels by A = max|v|*max|k|; the hardest 128 per core go through
an exact pair-grid tile.

Performance structure:
  - R is produced directly by Lc-weighted reduction stationaries (no
    separate Lc matmul or That round-trip through SBUF).
  - chebyshev chain parity-split over z = 2*khat^2-1: even/odd subchains
    halve the serial dependency depth on the vector engine.
  - PE warmup matmuls + junk filler (written to unused partition rows of
    the reduce psum tile) keep the HAM clock gate at 2.4 GHz through the
    analysis and S phases.
  - eval: per-node S broadcast (PE, full stationary) -> fp16 SBUF copy
    (ACT) -> 2x DVE multiply -> group-split identity-accumulate matmul
    pairs; a few nodes read PSUM directly on DVE to balance engines.
  - the exact tile's heavy ops are emitted mid-eval so its activation
    passes fill scalar-engine slack instead of blocking the S->eval
    transition.
  - no Ln activations (reciprocal_approx_fast): one exp table load.
"""

import sys

sys.path.insert(0, "/opt/trn_rl_repo")

from contextlib import ExitStack

import ml_dtypes
import numpy as np

import concourse.bacc as bacc
import concourse.bass as bass
import concourse.tile as tile
from concourse import mybir
from concourse.bass_utils import run_bass_kernel_spmd

B, C, H, W = 2, 64, 128, 128
N_CORES = 8
NPIX = B * H * W            # 32768
M = 11                      # interp nodes (odd: center node u=0 is free)
MC = M // 2
NEX_CORE = 128              # exact pixels per core
NEZ_CORE = NPIX // N_CORES - NEX_CORE   # 3968 interp pixels per core
FD = NEZ_CORE // 2          # 1984 pixels per g-half
HFD = FD // 2               # 992 pixels per column-half
R2 = 2 * M
N_WARM = 20                 # PE warmup matmuls
N_JUNK_ANA = 4              # filler matmuls per analysis m-step
# (half, m) slots whose S stays in PSUM (DVE 1x mul, no ACT copy)
PSUM_MUL_SLOTS = {(0, 2), (1, 2), (0, 8), (1, 8)}

FP32 = mybir.dt.float32
FP16 = mybir.dt.float16
BF16 = mybir.dt.bfloat16
EXP = mybir.ActivationFunctionType.Exp

U_NODES = np.cos(np.pi * np.arange(M) / (M - 1))


def _lc_matrix():
    u = U_NODES
    Tn = np.cos(np.arange(M)[:, None] * np.arccos(np.clip(u, -1, 1))[None, :])
    return np.linalg.inv(Tn.T)


def build_kernel():
    nc = bacc.Bacc(
        "TRN2",
        target_bir_lowering=False,
        debug=False,
        enable_asserts=False,
        num_devices=N_CORES,
    )
    vp = nc.dram_tensor("vp", [128, FD], FP16, kind="ExternalInput").ap()
    k2t = nc.dram_tensor("k2t", [128, FD], FP16, kind="ExternalInput").ap()
    qt = nc.dram_tensor("qt", [128, FD], FP16, kind="ExternalInput").ap()
    u1t = nc.dram_tensor("u1t", [128, FD], FP16, kind="ExternalInput").ap()
    statR = nc.dram_tensor("statR", [128, M, R2], FP16, kind="ExternalInput").ap()
    statL = nc.dram_tensor("statL", [128, M, R2], FP16, kind="ExternalInput").ap()
    statB = nc.dram_tensor("statB", [R2, M, 128], FP16, kind="ExternalInput").ap()
    identb = nc.dram_tensor("identb", [128, 128], BF16, kind="ExternalInput").ap()
    ident2 = nc.dram_tensor("ident2", [128, 64], FP16, kind="ExternalInput").ap()
    vE = nc.dram_tensor("vE", [128, C], FP32, kind="ExternalInput").ap()
    kE = nc.dram_tensor("kE", [128, C], FP32, kind="ExternalInput").ap()
    qE = nc.dram_tensor("qE", [128, C], FP32, kind="ExternalInput").ap()

    outm = nc.dram_tensor("outm", [128, FD], FP16, kind="ExternalOutput").ap()
    oute = nc.dram_tensor("oute", [128, C], FP32, kind="ExternalOutput").ap()

    CH = [0, 512, HFD]

    with tile.TileContext(nc) as tc, ExitStack() as ctx:
        sb = ctx.enter_context(tc.tile_pool(name="sb", bufs=1))
        sbw = ctx.enter_context(tc.tile_pool(name="sbw", bufs=1))
        sbp = ctx.enter_context(tc.tile_pool(name="sbp", bufs=2))
        acp = ctx.enter_context(tc.tile_pool(name="acp", bufs=2, space="PSUM"))

        # ---- PE warmup: junk matmuls from t~0 flip HAM to 2.4 GHz ----
        wu = sbw.tile([128, 512], FP16, tag="wu")
        nc.vector.memset(wu, 1.0)
        wu_ps = acp.tile([128, HFD], FP32, tag="acc", name="wups")
        for i in range(N_WARM):
            nc.tensor.matmul(wu_ps[:, 0:496], wu[:, 0:128], wu[:, 0:496],
                             start=True, stop=True)
        accs = [acp.tile([128, HFD], FP32, tag="acc", name=f"acc{h}")
                for h in range(2)]
        junk_tgts = []
        _junk_i = [0]

        def junk_mm(n):
            # filler matmuls into unused partition rows of live psum tiles
            # (same bank, disjoint partitions): keeps the PE activity
            # monitor busy so the clock gate stays at 2.4 GHz
            for _ in range(n):
                t = junk_tgts[_junk_i[0] % len(junk_tgts)]
                _junk_i[0] += 1
                nc.tensor.matmul(t[64:128, 0:496], wu[:, 0:64], wu[:, 0:496],
                                 start=True, stop=True, skip_group_check=True)

        # dummy activation: pull the exp table load off the critical path
        dum = sbw.tile([1, 16], FP32, tag="dum")
        nc.scalar.activation(out=dum, in_=wu[0:1, 0:16], func=EXP)

        # ---- input DMAs ----
        v_t = sb.tile([128, FD], FP16)
        k2_t = sb.tile([128, FD], FP16)
        q_t = sb.tile([128, FD], FP16)
        u1_t = sb.tile([128, FD], FP16)
        sR = sb.tile([128, M, R2], FP16)
        sL = sb.tile([128, M, R2], FP16)
        sB_t = sb.tile([R2, M, 128], FP16)
        id_t = sb.tile([128, 128], BF16)
        id2_t = sb.tile([128, 64], FP16)
        nc.sync.dma_start(out=v_t[:, :FD // 2], in_=vp[:, :FD // 2])
        nc.sync.dma_start(out=k2_t, in_=k2t)
        nc.sync.dma_start(out=u1_t, in_=u1t)
        nc.sync.dma_start(out=q_t, in_=qt)
        nc.sync.dma_start(out=v_t[:, FD // 2:], in_=vp[:, FD // 2:])
        nc.gpsimd.dma_start(out=sR, in_=statR)
        nc.gpsimd.dma_start(out=sL, in_=statL)
        nc.gpsimd.dma_start(out=sB_t, in_=statB)
        nc.gpsimd.dma_start(out=id_t, in_=identb)
        nc.gpsimd.dma_start(out=id2_t, in_=ident2)

        ones_t = sbw.tile([128, FD], FP16, tag="ones")
        nc.vector.memset(ones_t, 1.0)

        # exact-tile small inputs (early: cheap, off the critical path)
        vE_t = sb.tile([128, C], FP32)
        nc.sync.dma_start(out=vE_t, in_=vE)
        kE_t = sb.tile([128, C], FP32)
        nc.sync.dma_start(out=kE_t, in_=kE)
        qE_t = sb.tile([128, C], FP32)
        nc.sync.dma_start(out=qE_t, in_=qE)
        v2 = sb.tile([128, C, 2], FP16)
        nc.scalar.copy(v2, vE_t[:, :, None].broadcast_to([128, C, 2]))
        kE16 = sb.tile([128, C], FP16)
        nc.scalar.copy(kE16, kE_t)

        # ---- X grids (ACT, fp16), per column-half for early start ----
        Xs = {}
        for m in range(M):
            if m == MC:
                Xs[m] = ones_t
                continue
            xm = sbw.tile([128, FD], FP16, tag=f"x{m}")
            nc.scalar.activation(out=xm[:, :FD // 2], in_=v_t[:, :FD // 2],
                                 func=EXP, scale=float(U_NODES[m]))
            nc.scalar.activation(out=xm[:, FD // 2:], in_=v_t[:, FD // 2:],
                                 func=EXP, scale=float(U_NODES[m]))
            Xs[m] = xm

        # ---- chebyshev chain, parity split over zz = 2z = k2^2 - 2 ----
        # U_r = T_r(khat) * q.  evens: E_s = T_s(z) q = U_{2s};
        # odds: O_s = cos((2s+1)theta) q = U_{2s+1};
        # both satisfy  next = zz * cur - prev.
        zz = sbw.tile([128, FD], FP16, tag="zz")
        nc.vector.tensor_mul(zz, k2_t, k2_t)
        nc.vector.tensor_scalar_add(zz, zz, -2.0)
        Us = {0: q_t, 1: u1_t}
        e1 = sbw.tile([128, FD], FP16, tag="e1")
        nc.vector.scalar_tensor_tensor(
            out=e1, in0=zz, scalar=0.5, in1=q_t,
            op0=mybir.AluOpType.mult, op1=mybir.AluOpType.mult)
        Us[2] = e1
        o1 = sbw.tile([128, FD], FP16, tag="o1")
        nc.vector.scalar_tensor_tensor(
            out=o1, in0=zz, scalar=-1.0, in1=u1_t,
            op0=mybir.AluOpType.add, op1=mybir.AluOpType.mult)
        Us[3] = o1
        for par in (0, 1):
            prev, cur = Us[0 + par], Us[2 + par]
            for r in range(4 + par, M, 2):
                tmp = sbp.tile([128, FD], FP16, tag=f"tmp{par}")
                nc.vector.tensor_mul(tmp, zz, cur)
                nxt = sbw.tile([128, FD], FP16, tag=f"u{r}")
                nc.vector.tensor_sub(nxt, tmp, prev)
                Us[r] = nxt
                prev, cur = cur, nxt

        # ---- analysis reductions: R rows 0:22 (Lc-weighted), G rows 32:54
        with tc.tile_pool(name="red", bufs=2, space="PSUM") as redp:
            red_ts = [redp.tile([128, HFD], FP32, tag="red", name=f"red{h}")
                      for h in range(2)]
            junk_tgts.extend(red_ts)
            for m in range(M):
                for h in range(2):
                    hs = slice(h * HFD, (h + 1) * HFD)
                    r_ps = red_ts[h][0:R2, :]
                    g_ps = red_ts[h][32:32 + R2, :]
                    for a, b in zip(CH[:-1], CH[1:]):
                        nc.tensor.matmul(r_ps[:, a:b], sL[:, m, :],
                                         Us[m][:, hs][:, a:b],
                                         start=(m == 0), stop=(m == M - 1))
                        nc.tensor.matmul(g_ps[:, a:b], sR[:, m, :],
                                         Xs[m][:, hs][:, a:b],
                                         start=(m == 0), stop=(m == M - 1))
                junk_mm(N_JUNK_ANA)

            # ---- S = R / G per half ----
            s_halves = []
            for h in range(2):
                r_ps = red_ts[h][0:R2, :]
                g_ps = red_ts[h][32:32 + R2, :]
                gsb = sbw.tile([R2, HFD], FP32, tag=f"gsb{h}")
                nc.scalar.copy(gsb, g_ps)
                junk_mm(7)
                ginv = sbw.tile([R2, HFD], FP32, tag=f"ginv{h}")
                nc.vector.reciprocal_approx_fast(out=ginv, in_=gsb)
                s_th = sbw.tile([R2, HFD], FP16, tag=f"s{h}")
                nc.vector.tensor_mul(s_th, ginv, r_ps)
                s_halves.append(s_th)
                junk_mm(4)

        # ---- eval + exact tile woven together ----
        with tc.tile_pool(name="evp", bufs=2, space="PSUM") as evp:

            def eval_slot(m, half):
                acc = accs[half]
                s_h = s_halves[half]
                if m == MC:
                    for a, b in zip(CH[:-1], CH[1:]):
                        nc.tensor.matmul(acc[0:64, a:b], sB_t[:, m, 0:64],
                                         s_h[:, a:b], start=False, stop=False,
                                         skip_group_check=True)
                        nc.tensor.matmul(acc[64:128, a:b], sB_t[:, m, 64:128],
                                         s_h[:, a:b], start=False, stop=False,
                                         skip_group_check=True)
                    return
                s_b = evp.tile([128, HFD], FP32, tag="sbps",
                               name=f"sb{half}_{m}")
                for a, b in zip(CH[:-1], CH[1:]):
                    nc.tensor.matmul(s_b[:, a:b], sB_t[:, m, :],
                                     s_h[:, a:b], start=True, stop=True)
                prod = sbp.tile([128, HFD], FP16, tag="prod", bufs=4)
                if (half, m) in PSUM_MUL_SLOTS:
                    nc.vector.tensor_mul(
                        prod, Xs[m][:, half * HFD:(half + 1) * HFD], s_b)
                else:
                    s_bs = sbp.tile([128, HFD], FP16, tag="sbs", bufs=4)
                    nc.scalar.copy(s_bs, s_b)
                    nc.vector.tensor_mul(
                        prod, Xs[m][:, half * HFD:(half + 1) * HFD], s_bs)
                st = (m == 0)
                sp = (m == M - 1)
                for a, b in zip(CH[:-1], CH[1:]):
                    nc.tensor.matmul(acc[0:64, a:b], id2_t[0:64, :],
                                     prod[0:64, a:b], start=st, stop=sp,
                                     skip_group_check=True)
                    nc.tensor.matmul(acc[64:128, a:b], id2_t[64:128, :],
                                     prod[64:128, a:b], start=st, stop=sp,
                                     skip_group_check=True,
                                     tile_position=(64, 64))

            for m in range(3):
                for half in range(2):
                    eval_slot(m, half)

            # exact tile emitted here: its ACT/DVE work fills engine slack
            # during the eval stream instead of blocking the S->eval start
            P_t = sb.tile([128, C, C], FP16)
            k_op = bass.AP(
                tensor=kE16.tensor, offset=kE16.offset,
                ap=[kE16.ap[0], [0, C], [2, C // 2], [1, 2]],
            )
            v_op = bass.AP(
                tensor=v2.tensor, offset=v2.offset,
                ap=[v2.ap[0], [2, C], [0, C // 2], [1, 2]],
            )
            nc.vector.tensor_mul(
                P_t.rearrange("p i (jh jp) -> p i jh jp", jp=2), k_op, v_op)
            E_t = sb.tile([128, C, C], BF16)
            for eb in range(4):
                nc.scalar.activation(out=E_t[:, eb * 16:(eb + 1) * 16, :],
                                     in_=P_t[:, eb * 16:(eb + 1) * 16, :],
                                     func=EXP)
            G1 = sb.tile([128, C // 4, C], BF16)
            G2 = sb.tile([128, C // 4, C], BF16)
            nc.vector.tensor_add(G1, E_t[:, : C // 4, :], E_t[:, C // 4: C // 2, :])
            nc.vector.tensor_add(G2, E_t[:, C // 2: 3 * C // 4, :], E_t[:, 3 * C // 4:, :])
            nc.gpsimd.dma_start(out=G1, in_=G2, accum_op=mybir.AluOpType.add)

            for m in range(3, 7):
                for half in range(2):
                    eval_slot(m, half)

            nc.vector.tensor_add(G1[:, : C // 8, :], G1[:, : C // 8, :],
                                 G1[:, C // 8: C // 4, :])
            nc.vector.tensor_add(G1[:, : C // 16, :], G1[:, : C // 16, :],
                                 G1[:, C // 16: C // 8, :])
            d_t = sb.tile([128, C], FP32)
            nc.vector.tensor_reduce(
                out=d_t, in_=G1[:, : C // 16, :].transpose([0, 2, 1]),
                axis=mybir.AxisListType.X, op=mybir.AluOpType.add,
            )
            r_t = sb.tile([128, C], FP32)
            nc.vector.reciprocal_approx_fast(out=r_t, in_=d_t)
            w16 = sb.tile([128, C], BF16)
            nc.vector.tensor_mul(w16, qE_t, r_t)
            Q4 = C // 4
            F1 = sb.tile([128, C, Q4], BF16)
            F2 = sb.tile([128, C, Q4], BF16)
            F3 = sb.tile([128, C, Q4], BF16)
            F4 = sb.tile([128, C, Q4], BF16)
            for fi, Fq in enumerate((F1, F2, F3, F4)):
                nc.vector.tensor_mul(
                    Fq, E_t[:, :, fi * Q4: (fi + 1) * Q4],
                    w16[:, None, fi * Q4: (fi + 1) * Q4].broadcast_to([128, C, Q4]),
                )
            nc.gpsimd.dma_start(out=F1, in_=F2, accum_op=mybir.AluOpType.add)
            nc.gpsimd.dma_start(out=F3, in_=F4, accum_op=mybir.AluOpType.add)

            for m in range(7, M):
                for half in range(2):
                    eval_slot(m, half)

            nc.vector.tensor_add(F1, F1, F3)
            nc.vector.tensor_add(F1[:, :, : Q4 // 2], F1[:, :, : Q4 // 2],
                                 F1[:, :, Q4 // 2:])
            nc.vector.tensor_add(F1[:, :, : Q4 // 4], F1[:, :, : Q4 // 4],
                                 F1[:, :, Q4 // 4: Q4 // 2])
            oE = sb.tile([128, C], FP32)
            nc.vector.tensor_reduce(
                out=oE, in_=F1[:, :, : Q4 // 4],
                axis=mybir.AxisListType.X, op=mybir.AluOpType.add,
            )
            nc.sync.dma_start(out=oute, in_=oE)

            for half in range(2):
                o_sb = sbp.tile([128, HFD], FP16, tag="osb")
                nc.scalar.copy(o_sb, accs[half][:, 0:HFD])
                nc.sync.dma_start(
                    out=outm[:, half * HFD:(half + 1) * HFD], in_=o_sb)

    nc.compile()
    return nc


_NC_CACHE = None


def _get_nc():
    global _NC_CACHE
    if _NC_CACHE is None:
        _NC_CACHE = build_kernel()
    return _NC_CACHE


def _prep(x, y, z):
    """Host prep: sort by difficulty, shard, scale. Returns in_maps + meta."""
    q = np.ascontiguousarray(np.transpose(np.asarray(x), (0, 2, 3, 1))).reshape(-1, C)
    k = np.ascontiguousarray(np.transpose(np.asarray(y), (0, 2, 3, 1))).reshape(-1, C)
    v = np.ascontiguousarray(np.transpose(np.asarray(z), (0, 2, 3, 1))).reshape(-1, C)
    Tk = np.abs(k).max(axis=1)
    A = Tk * np.abs(v).max(axis=1)
    order = np.argsort(A, kind="stable")
    easy = order[: NEZ_CORE * N_CORES]
    hard = order[NEZ_CORE * N_CORES:]

    Lc = _lc_matrix()
    statR = np.zeros((128, M, R2), np.float32)
    for m in range(M):
        for g in range(2):
            statR[g * 64:(g + 1) * 64, m, 2 * m + g] = 1
    statL = np.zeros((128, M, R2), np.float32)
    for r in range(M):
        for m in range(M):
            for g in range(2):
                statL[g * 64:(g + 1) * 64, r, 2 * m + g] = Lc[r, m]
    statB = np.zeros((R2, M, 128), np.float32)
    for m in range(M):
        for g in range(2):
            statB[2 * m + g, m, g * 64:(g + 1) * 64] = 1
    identb = np.eye(128, dtype=np.float32)
    ident2 = np.zeros((128, 64), np.float32)
    ident2[0:64] = np.eye(64)
    ident2[64:128] = np.eye(64)

    in_maps = []
    meta = []
    for c in range(N_CORES):
        ez = easy[c::N_CORES]
        hd = hard[c::N_CORES]
        kh = k[ez] / Tk[ez, None]
        vp_c = (Tk[ez, None] * v[ez]).astype(np.float16)
        k2_c = (2.0 * kh).astype(np.float16)
        q_c = q[ez].astype(np.float16)
        u1_c = (kh * q[ez]).astype(np.float16)

        def cmaj(a2d, dt):
            h0 = a2d[:FD].T
            h1 = a2d[FD:].T
            return np.ascontiguousarray(np.concatenate([h0, h1], axis=0)).astype(dt)

        in_maps.append({
            "vp": cmaj(vp_c, np.float16),
            "k2t": cmaj(k2_c, np.float16),
            "qt": cmaj(q_c, np.float16),
            "u1t": cmaj(u1_c, np.float16),
            "statR": statR.astype(np.float16),
            "statL": statL.astype(np.float16),
            "statB": statB.astype(np.float16),
            "identb": identb.astype(ml_dtypes.bfloat16),
            "ident2": ident2.astype(np.float16),
            "vE": v[hd].astype(np.float32),
            "kE": k[hd].astype(np.float32),
            "qE": q[hd].astype(np.float32),
        })
        meta.append((ez, hd))
    return in_maps, meta


def kernel(x, y, z):
    nc = _get_nc()
    in_maps, meta = _prep(x, y, z)
    res = run_bass_kernel_spmd(nc, in_maps, core_ids=list(range(N_CORES)))
    out = np.empty((NPIX, C), np.float32)
    for c in range(N_CORES):
        ez, hd = meta[c]
        om = res.results[c]["outm"].astype(np.float32)
        out[ez[:FD]] = om[:64].T
        out[ez[FD:]] = om[64:].T
        out[hd] = res.results[c]["oute"]
    return np.ascontiguousarray(
        np.transpose(out.reshape(B, H, W, C), (0, 3, 1, 2))
    ).astype(np.float32)
